# revision 1
# baseline (speedup 1.0000x reference)
"""CrossNetMix (FuxiCTR MoE-routing) Trainium2 Bass kernel.

Math: the reference updates Xi = Xi + X0 * xw with xw of shape (B, 1), so
Xi is always a per-row scalar multiple of X0: Xi = c_b * X0[b].  With
precomputed per-row projections of X0 (g0 = X0@Wg^T, p0[l,e] =
sum_r (X0@U^T)(X0@V^T)), each layer reduces to a tiny per-row scalar
recurrence:

    gate_logits = c * g0[l] + bg[l]
    xw          = c^2 * sum_e(p0[l] * softmax(gate_logits))
    c          += xw

so the whole network is ONE fused matmul X0 @ W_all^T + an epilogue.

Product trick: with A = (U+V)/2, B = (U-V)/2 (packed host-side),
p0 = sum_r a^2 - sum_r b^2 where a = X0@A^T, b = X0@B^T.  The squares run
on the scalar (ACT) engine straight out of PSUM; only a segmented
reduce_sum runs on DVE.  This removes the PSUM->SBUF copy + elementwise
multiply of the naive u*v epilogue.

Sharding: data-parallel over batch across 8 NeuronCores; weights
replicated; no collectives.

Per-core schedule (Bc = 2048 rows = 16 m-panels of 128):
  - single phase: X^T panel blocks (host-transposed, plain 2D DMAs) are
    fully SBUF-resident (128KB/partition); the 6 A/B weight groups (512
    cols each, streamed once as k-half tiles through a 3-buffer pool) +
    gate never stall the PE.  PSUM pool = 3 bufs (empirically fastest;
    response to PSUM buffer count is strongly non-monotone on trn2).
  - matmul dtype float32r: full-rate (1 cyc/row) fp32 PE mode.
  - group order per phase: A0 B0 G A1 B1 A2 B2.  The layer-l scalar
    recurrence for panel m is emitted right after B_l's epilogue of that
    panel, and the output panel (c * X0 row-major) is finalized during
    the last group -- the post-matmul serial tail is ~one panel.
"""

import os
import numpy as np

import concourse.bacc as bacc
import concourse.mybir as mybir
from concourse.tile import TileContext
from concourse.bass_utils import run_bass_kernel_spmd

# Problem constants (hardcoded per contest contract)
B, D, L, E, R = 16384, 2048, 3, 8, 64
N_CORES = 8
BC = B // N_CORES  # 2048 rows per core
P = 128
KT = D // P        # 16 contraction tiles
MT = BC // P       # 16 m-panels per core
MPP = 16           # m-panels per phase (16 = single phase, W streamed once)
MPB = 4            # m-panels per xt block (prefetch granule)
GW = E * R         # 512 = A/B group width (one layer, all experts)
N_UV = 2 * L       # 6 A/B groups
GATE = L * E       # 24 gate columns

_F32 = mybir.dt.float32


def build_nc(mm_dtype=mybir.dt.float32r, mt: int = MT, mpp: int = MPP,
             reps: int = 1):
    """Build the per-core Bass kernel. mt<MT builds a reduced-size kernel
    (for simulation); reps>1 wraps the body in a hardware loop (timing).
    KSTAGE env: 1=matmuls only, 2=+epilogue, 3=full (default)."""
    stage = int(os.environ.get("KSTAGE", "3"))
    nc = bacc.Bacc("TRN2", target_bir_lowering=False, debug=False,
                   num_devices=N_CORES)
    bc = mt * P
    mpp = int(os.environ.get("MPP_OVR", mpp))
    whalf = os.environ.get("WHALF", "1") == "1"
    sqip = os.environ.get("SQIP", "0") == "1"
    xb = int(os.environ.get("XB", "4"))
    wb = int(os.environ.get("WB", "3"))
    mpp = min(mpp, mt)
    n_ph = (mt + mpp - 1) // mpp
    mpb = min(MPB, mpp)
    bpp = (mpp + mpb - 1) // mpb  # xt blocks per phase

    x0 = nc.dram_tensor("X0", [bc, D], _F32, kind="ExternalInput")
    xtb = nc.dram_tensor("XTB", [mt, P, KT * P], mm_dtype,
                         kind="ExternalInput")
    wab = nc.dram_tensor("WAB", [N_UV, P, KT * GW], mm_dtype,
                         kind="ExternalInput")
    wgd = nc.dram_tensor("WG", [P, KT * GATE], mm_dtype,
                         kind="ExternalInput")
    bgr = nc.dram_tensor("BG", [P, GATE], _F32, kind="ExternalInput")
    out = nc.dram_tensor("OUT", [bc, D], _F32, kind="ExternalOutput")

    # group sequence: (kind, layer); gate after B0 so the layer-0
    # recurrence can start, A/B of later layers after it.
    seq = [("A", 0), ("B", 0), ("G", 0), ("A", 1), ("B", 1),
           ("A", 2), ("B", 2)]

    with TileContext(nc) as tc:
        with (
            tc.tile_pool(name="xt_p", bufs=xb) as xt_pool,
            tc.tile_pool(name="w_p", bufs=wb) as w_pool,
            tc.tile_pool(name="wg_p", bufs=1) as wg_pool,
            tc.tile_pool(name="pg_p", bufs=1) as pg_pool,
            tc.tile_pool(name="sq_p", bufs=3) as sq_pool,
            tc.tile_pool(name="sm_p", bufs=2) as sm_pool,
            tc.tile_pool(name="xp_p", bufs=2) as xp_pool,
            tc.tile_pool(name="ps_p", bufs=int(os.environ.get("PSB", "3")),
                         space="PSUM") as ps_pool,
        ):
            # --- persistent tiles ---
            wg_sb = wg_pool.tile([P, KT * GATE], mm_dtype, tag="wg")
            bg_sb = wg_pool.tile([P, GATE], _F32, tag="bg")
            # per panel m: [pA0 pB0 pA1 pB1 pA2 pB2 (8 each) | g0 (24)]
            pg_sb = pg_pool.tile([P, mt * 72], _F32, tag="pg")
            c_sb = wg_pool.tile([P, mt], _F32, tag="c")

            nc.scalar.dma_start(out=wg_sb[:], in_=wgd[:])
            nc.scalar.dma_start(out=bg_sb[:], in_=bgr[:])

            def recurrence(m, l):
                c_m = c_sb[:, m : m + 1]
                base = m * 72
                pa = pg_sb[:, base + 2 * l * 8 : base + 2 * l * 8 + 8]
                pb = pg_sb[:, base + (2 * l + 1) * 8 : base + (2 * l + 2) * 8]
                g0l = pg_sb[:, base + 48 + l * E : base + 48 + (l + 1) * E]
                d = sm_pool.tile([P, E], _F32, tag="d")
                t = sm_pool.tile([P, E], _F32, tag="t")
                et = sm_pool.tile([P, E], _F32, tag="et")
                nmx = sm_pool.tile([P, 1], _F32, tag="nmx")
                s1 = sm_pool.tile([P, 1], _F32, tag="s1")
                s2 = sm_pool.tile([P, 1], _F32, tag="s2")
                rcp = sm_pool.tile([P, 1], _F32, tag="rcp")
                e1 = sm_pool.tile([P, 1], _F32, tag="e1")
                # d = pA - pB  (= p0[l])
                nc.vector.tensor_tensor(d[:], pa, pb,
                                        op=mybir.AluOpType.subtract)
                # t = c * g0[l] + bg[l]
                nc.vector.scalar_tensor_tensor(
                    t[:], g0l, c_m, bg_sb[:, l * E : (l + 1) * E],
                    op0=mybir.AluOpType.mult, op1=mybir.AluOpType.add,
                )
                # nmx = -max_e t
                nc.vector.tensor_reduce(
                    nmx[:], t[:], axis=mybir.AxisListType.X,
                    op=mybir.AluOpType.max, negate=True,
                )
                # et = exp(t - max); s2 = sum_e et
                nc.scalar.activation(
                    et[:], t[:], mybir.ActivationFunctionType.Exp,
                    bias=nmx[:], scale=1.0, accum_out=s2[:],
                )
                # s1 = sum_e d * et
                nc.vector.scalar_tensor_tensor(
                    t[:], d[:], 1.0, et[:],
                    op0=mybir.AluOpType.mult, op1=mybir.AluOpType.mult,
                    accum_out=s1[:],
                )
                nc.vector.reciprocal(rcp[:], s2[:])
                # e1 = s1 * rcp * c ; c += e1 * c
                nc.vector.scalar_tensor_tensor(
                    e1[:], s1[:], rcp[:], c_m,
                    op0=mybir.AluOpType.mult, op1=mybir.AluOpType.mult,
                )
                nc.vector.scalar_tensor_tensor(
                    c_m, e1[:], c_m, c_m,
                    op0=mybir.AluOpType.mult, op1=mybir.AluOpType.add,
                )

            def body(_iv=None):
                xp_tiles = {}
                nc.vector.memset(c_sb[:], 1.0)
                for ph in range(n_ph):
                    lo = ph * mpp
                    npan = min(mpp, mt - lo)
                    # phase-resident X^T blocks (plain 2D DMAs per panel)
                    blks = []
                    for b in range(bpp):
                        xt_sb = xt_pool.tile([P, mpb * KT * P], mm_dtype,
                                             tag="xt")
                        blks.append(xt_sb)
                        for j in range(mpb):
                            mloc = b * mpb + j
                            if mloc >= npan:
                                break
                            nc.gpsimd.dma_start(
                                out=xt_sb[:, j * KT * P : (j + 1) * KT * P],
                                in_=xtb[lo + mloc],
                            )

                    def xt_panel(mloc, k):
                        t = blks[mloc // mpb]
                        j = mloc % mpb
                        base = (j * KT + k) * P
                        return t[:, base : base + P]

                    for kind, l in seq:
                        if kind == "G":
                            ncols = GATE

                            def wslice(k):
                                return wg_sb[:, k * GATE : (k + 1) * GATE]
                        elif whalf:
                            g = 2 * l + (kind == "B")
                            hw = KT * GW // 2
                            whs = []
                            for h in range(2):
                                wh = w_pool.tile([P, hw], mm_dtype, tag="w")
                                nc.scalar.dma_start(
                                    out=wh[:],
                                    in_=wab[g][:, h * hw : (h + 1) * hw],
                                )
                                whs.append(wh)
                            ncols = GW

                            def wslice(k, whs=whs):
                                t = whs[k // (KT // 2)]
                                kk = k % (KT // 2)
                                return t[:, kk * GW : (kk + 1) * GW]
                        else:
                            g = 2 * l + (kind == "B")
                            w_sb = w_pool.tile([P, KT * GW], mm_dtype,
                                               tag="w")
                            nc.scalar.dma_start(out=w_sb[:], in_=wab[g])
                            ncols = GW

                            def wslice(k, w_sb=w_sb):
                                return w_sb[:, k * GW : (k + 1) * GW]
                        last = kind == "B" and l == L - 1
                        for mloc in range(npan):
                            m = lo + mloc
                            if stage >= 3 and kind == "A" and l == L - 1:
                                # prefetch X0 panel for the finalize, one
                                # group ahead of its use in B2
                                xp = xp_pool.tile([P, D], _F32, tag="xp")
                                xp_tiles[m] = xp
                                nc.gpsimd.dma_start(
                                    out=xp[:],
                                    in_=x0[m * P : (m + 1) * P, :],
                                )
                            ps = ps_pool.tile([P, GW], _F32, tag="ps")
                            for k in range(KT):
                                nc.tensor.matmul(
                                    ps[:, :ncols],
                                    xt_panel(mloc, k),
                                    wslice(k),
                                    start=(k == 0),
                                    stop=(k == KT - 1),
                                )
                            base = m * 72
                            if kind == "G":
                                if stage >= 2:
                                    nc.vector.tensor_copy(
                                        pg_sb[:, base + 48 : base + 72],
                                        ps[:, :GATE],
                                    )
                                if stage >= 3:
                                    recurrence(m, 0)
                            else:
                                if stage >= 2:
                                    if sqip:
                                        # square in place in PSUM, reduce
                                        # straight from PSUM
                                        sq = ps
                                    else:
                                        sq = sq_pool.tile([P, GW], _F32,
                                                          tag="sq")
                                    nc.scalar.activation(
                                        sq[:], ps[:],
                                        mybir.ActivationFunctionType.Square,
                                    )
                                    slot = base + (2 * l + (kind == "B")) * 8
                                    nc.vector.reduce_sum(
                                        pg_sb[:, slot : slot + 8],
                                        sq[:].rearrange(
                                            "p (e r) -> p e r", e=E
                                        ),
                                        axis=mybir.AxisListType.X,
                                    )
                                if stage >= 3 and kind == "B" and l >= 1:
                                    recurrence(m, l)
                                if stage >= 3 and last:
                                    # out panel = c * X0 panel
                                    xp = xp_tiles.pop(m)
                                    c_m = c_sb[:, m : m + 1]
                                    nc.vector.tensor_scalar_mul(
                                        xp[:], xp[:], c_m
                                    )
                                    nc.sync.dma_start(
                                        out=out[m * P : (m + 1) * P, :],
                                        in_=xp[:],
                                    )
                                if stage < 3 and last and mloc == npan - 1:
                                    fl = xp_pool.tile([P, GW], _F32,
                                                      tag="flush")
                                    nc.vector.tensor_copy(fl[:], ps[:])
                                    nc.sync.dma_start(
                                        out=out[0:P, :GW], in_=fl[:]
                                    )

            if reps == 1:
                body()
            else:
                with tc.For_i(0, reps, 1) as iv:
                    body(iv)

    nc.compile()
    return nc


def pack_weights(U, V, Wg):
    """Host-side packing: A/B groups + gate, laid out so every device DMA
    is a plain contiguous [128, N] transfer."""
    A = (U + V) * 0.5  # (L, E, R, D)
    Bm = (U - V) * 0.5
    allw = np.empty((N_UV, GW, D), np.float32)
    for l in range(L):
        allw[2 * l] = A[l].reshape(GW, D)
        allw[2 * l + 1] = Bm[l].reshape(GW, D)
    # [g, c, k, p] -> [g, p, k, c]
    wab = np.ascontiguousarray(
        allw.reshape(N_UV, GW, KT, P).transpose(0, 3, 2, 1)
    ).reshape(N_UV, P, KT * GW)
    wg = np.ascontiguousarray(
        Wg.reshape(GATE, KT, P).transpose(2, 1, 0)
    ).reshape(P, KT * GATE)
    return wab, wg


def pack_xtb(x0_shard, mt=MT):
    """[bc, D] -> [mt, P(d-within-k), KT*P(b-within-panel)] blocked
    transpose so each panel is one contiguous [128, 2048] DMA."""
    return np.ascontiguousarray(
        x0_shard.reshape(mt, P, KT, P).transpose(0, 3, 2, 1)
    ).reshape(mt, P, KT * P)


def make_in_maps(X0, U, V, Wg, bg):
    X0 = np.ascontiguousarray(np.asarray(X0, dtype=np.float32))
    wab, wg = pack_weights(
        np.asarray(U, np.float32), np.asarray(V, np.float32),
        np.asarray(Wg, np.float32)
    )
    bg_rep = np.ascontiguousarray(
        np.broadcast_to(np.asarray(bg, np.float32).reshape(1, GATE),
                        (P, GATE))
    )
    in_maps = []
    for c in range(N_CORES):
        sh = X0[c * BC : (c + 1) * BC]
        in_maps.append(
            {
                "X0": sh,
                "XTB": pack_xtb(sh),
                "WAB": wab,
                "WG": wg,
                "BG": bg_rep,
            }
        )
    return in_maps


_CACHE = {}


def _get_runner(mm_dtype_name: str):
    key = mm_dtype_name
    if key not in _CACHE:
        _CACHE[key] = build_nc(getattr(mybir.dt, mm_dtype_name))
    return _CACHE[key]


def kernel(X0, U, V, Wg, bg):
    in_maps = make_in_maps(X0, U, V, Wg, bg)
    mm_dtype_name = os.environ.get("KERNEL_MM_DTYPE", "float32r")
    nc = _get_runner(mm_dtype_name)
    res = run_bass_kernel_spmd(nc, in_maps, list(range(N_CORES)))
    return np.concatenate(
        [res.results[c]["OUT"] for c in range(N_CORES)], axis=0
    )



# revision 10
# speedup vs baseline: 1.2586x; 1.2586x over previous
"""CrossNetMix (FuxiCTR MoE-routing) Trainium2 Bass kernel.

Math: the reference updates Xi = Xi + X0 * xw with xw of shape (B, 1), so
Xi is always a per-row scalar multiple of X0: Xi = c_b * X0[b].  With
precomputed per-row projections of X0 (g0 = X0@Wg^T, p0[l,e] =
sum_r (X0@U^T)(X0@V^T)), each layer reduces to a tiny per-row scalar
recurrence:

    gate_logits = c * g0[l] + bg[l]
    xw          = c^2 * sum_e(p0[l] * softmax(gate_logits))
    c          += xw

so the whole network is ONE fused matmul X0 @ W_all^T + an epilogue.

Product trick: with A = (U+V)/2, B = (U-V)/2 (packed host-side),
p0 = sum_r a^2 - sum_r b^2 where a = X0@A^T, b = X0@B^T.  The squares run
on the scalar (ACT) engine straight out of PSUM; only a segmented
reduce_sum runs on DVE.  This removes the PSUM->SBUF copy + elementwise
multiply of the naive u*v epilogue.

Sharding: data-parallel over batch across 8 NeuronCores; weights
replicated; no collectives.

Per-core schedule (Bc = 2048 rows = 16 m-panels of 128):
  - single phase: X^T panel blocks (host-transposed, plain 2D DMAs) are
    fully SBUF-resident (128KB/partition); the 6 A/B weight groups (512
    cols each, streamed once as k-half tiles through a 3-buffer pool) +
    gate never stall the PE.  PSUM pool = 3 bufs (empirically fastest;
    response to PSUM buffer count is strongly non-monotone on trn2).
  - matmul dtype float32r: full-rate (1 cyc/row) fp32 PE mode.
  - group order per phase: A0 B0 G A1 B1 A2 B2.  The layer-l scalar
    recurrence for panel m is emitted right after B_l's epilogue of that
    panel, and the output panel (c * X0 row-major) is finalized during
    the last group -- the post-matmul serial tail is ~one panel.
"""

import os
import numpy as np

import concourse.bacc as bacc
import concourse.mybir as mybir
from concourse.tile import TileContext
from concourse.bass_utils import run_bass_kernel_spmd

# Problem constants (hardcoded per contest contract)
B, D, L, E, R = 16384, 2048, 3, 8, 64
N_CORES = 8
BC = B // N_CORES  # 2048 rows per core
P = 128
KT = D // P        # 16 contraction tiles
MT = BC // P       # 16 m-panels per core
MPP = 16           # m-panels per phase (16 = single phase, W streamed once)
MPB = 4            # m-panels per xt block (prefetch granule)
GW = E * R         # 512 = A/B group width (one layer, all experts)
N_UV = 2 * L       # 6 A/B groups
GATE = L * E       # 24 gate columns

_F32 = mybir.dt.float32


def dedup_ldweights(nc):
    """Remove InstLdweights whose stationary AP equals the previous
    ldweights in the same block's (already scheduled) PE stream.  Walrus
    lowers InstMatmult as non-self-loading (the paired InstLdweights does
    the load), so the PE array retains the stationary across matmuls —
    validated on HW (probe_dedup).  Only sync-free duplicates are
    removed; semaphore counts are unaffected (only matmuls carry sem
    updates)."""
    removed = 0
    pe = mybir.EngineType.PE
    for blk in nc.m.functions[0].blocks:
        insts = blk.instructions
        out = []
        last_key = None
        changed = False
        for inst in insts:
            tn = type(inst).__name__
            if tn == "InstLdweights":
                ap = inst.ins[0]
                key = (ap.memref, ap.offset, str(ap.ap), str(ap.dtype))
                si = inst.sync_info
                no_sync = si is None or (not si.on_wait and not si.on_update)
                if key == last_key and no_sync:
                    removed += 1
                    changed = True
                    continue
                last_key = key
            elif tn == "InstMatmult":
                pass  # does not clobber the loaded stationary
            elif getattr(inst, "engine", None) == pe and tn not in (
                "InstEventSemaphore",
            ):
                # unknown PE instruction: be conservative
                last_key = None
            out.append(inst)
        if changed:
            blk.instructions = out
    return removed


def build_nc_gi(mm_dtype=mybir.dt.float16, mt: int = MT, reps: int = 1):
    """Groups-inner schedule: per (panel, ktile) the X^T block is the
    stationary operand for 3-4 consecutive matmuls (one per weight
    group); redundant weight loads are removed post-schedule
    (dedup_ldweights).  All weights are SBUF-resident (fp16), streamed
    once per rep; X^T panels stream through a small pool.

    Per panel m: half0 = A0,B0,gate k-loop -> epilogue -> rec(m,0);
    half1 = A1,B1,A2,B2 k-loop -> epilogue -> rec(m,1), rec(m,2) ->
    finalize (OUT panel = c * X0 panel).  PSUM: 2+4 buffers of
    [128,512] rotate panel-to-panel; epilogues drain during the other
    half's matmuls, so the PE never waits on PSUM."""
    stage = int(os.environ.get("KSTAGE", "3"))
    xb = int(os.environ.get("XB", "4"))
    psb = int(os.environ.get("PSB", "6"))
    wchunks = int(os.environ.get("WCHUNKS", "4"))
    nc = bacc.Bacc("TRN2", target_bir_lowering=False, debug=False,
                   num_devices=N_CORES)
    bc = mt * P

    x0 = nc.dram_tensor("X0", [bc, D], _F32, kind="ExternalInput")
    xtb = nc.dram_tensor("XTB", [mt, P, KT * P], mm_dtype,
                         kind="ExternalInput")
    wab = nc.dram_tensor("WAB", [N_UV, P, KT * GW], mm_dtype,
                         kind="ExternalInput")
    wgd = nc.dram_tensor("WG", [P, KT * GATE], mm_dtype,
                         kind="ExternalInput")
    bgr = nc.dram_tensor("BG", [P, GATE], _F32, kind="ExternalInput")
    out = nc.dram_tensor("OUT", [bc, D], _F32, kind="ExternalOutput")

    halves = [[(0, "A"), (1, "B"), (None, "G")],
              [(2, "A"), (3, "B"), (4, "A"), (5, "B")]]
    # group index g -> (layer l, kind) for epilogue slot: g = 2l + (B)
    with TileContext(nc) as tc:
        with (
            tc.tile_pool(name="w_p", bufs=1) as w_pool,
            tc.tile_pool(name="xt_p", bufs=xb) as xt_pool,
            tc.tile_pool(name="xp_p", bufs=3) as xp_pool,
            tc.tile_pool(name="pg_p", bufs=1) as pg_pool,
            tc.tile_pool(name="sq_p", bufs=3) as sq_pool,
            tc.tile_pool(name="sm_p", bufs=2) as sm_pool,
            tc.tile_pool(name="ps_p", bufs=psb, space="PSUM") as ps_pool,
            tc.tile_pool(name="pg_ps", bufs=2, space="PSUM") as psg_pool,
        ):
            w_sb = []
            for g in range(N_UV):
                w_sb.append(w_pool.tile([P, KT * GW], mm_dtype,
                                        tag=f"w{g}", name=f"w{g}"))
            wg_sb = w_pool.tile([P, KT * GATE], mm_dtype, tag="wg")
            bg_sb = w_pool.tile([P, GATE], _F32, tag="bg")
            pg_sb = pg_pool.tile([P, mt * 72], _F32, tag="pg")
            c_sb = w_pool.tile([P, mt], _F32, tag="c")

            def weight_dmas():
                # spread across engine DMA queues, in consumption order:
                # half0 groups (w0, w1, gate) on ACT queue; half1 groups on
                # DVE + SP queues, chunked so early k-slices land first.
                cw = KT * GW // wchunks
                nc.scalar.dma_start(out=wg_sb[:], in_=wgd[:])
                nc.scalar.dma_start(out=bg_sb[:], in_=bgr[:])
                for ci in range(wchunks):
                    for g in (0, 1):
                        nc.scalar.dma_start(
                            out=w_sb[g][:, ci * cw : (ci + 1) * cw],
                            in_=wab[g][:, ci * cw : (ci + 1) * cw],
                        )
                    for g in (2, 3, 4, 5):
                        nc.sync.dma_start(
                            out=w_sb[g][:, ci * cw : (ci + 1) * cw],
                            in_=wab[g][:, ci * cw : (ci + 1) * cw],
                        )

            def recurrence(m, l):
                c_m = c_sb[:, m : m + 1]
                base = m * 72
                pa = pg_sb[:, base + 2 * l * 8 : base + 2 * l * 8 + 8]
                pb = pg_sb[:, base + (2 * l + 1) * 8 : base + (2 * l + 2) * 8]
                g0l = pg_sb[:, base + 48 + l * E : base + 48 + (l + 1) * E]
                d = sm_pool.tile([P, E], _F32, tag="d")
                t = sm_pool.tile([P, E], _F32, tag="t")
                et = sm_pool.tile([P, E], _F32, tag="et")
                nmx = sm_pool.tile([P, 1], _F32, tag="nmx")
                s1 = sm_pool.tile([P, 1], _F32, tag="s1")
                s2 = sm_pool.tile([P, 1], _F32, tag="s2")
                rcp = sm_pool.tile([P, 1], _F32, tag="rcp")
                e1 = sm_pool.tile([P, 1], _F32, tag="e1")
                nc.vector.tensor_tensor(d[:], pa, pb,
                                        op=mybir.AluOpType.subtract)
                nc.vector.scalar_tensor_tensor(
                    t[:], g0l, c_m, bg_sb[:, l * E : (l + 1) * E],
                    op0=mybir.AluOpType.mult, op1=mybir.AluOpType.add,
                )
                nc.vector.tensor_reduce(
                    nmx[:], t[:], axis=mybir.AxisListType.X,
                    op=mybir.AluOpType.max, negate=True,
                )
                nc.scalar.activation(
                    et[:], t[:], mybir.ActivationFunctionType.Exp,
                    bias=nmx[:], scale=1.0, accum_out=s2[:],
                )
                nc.vector.scalar_tensor_tensor(
                    t[:], d[:], 1.0, et[:],
                    op0=mybir.AluOpType.mult, op1=mybir.AluOpType.mult,
                    accum_out=s1[:],
                )
                nc.vector.reciprocal(rcp[:], s2[:])
                nc.vector.scalar_tensor_tensor(
                    e1[:], s1[:], rcp[:], c_m,
                    op0=mybir.AluOpType.mult, op1=mybir.AluOpType.mult,
                )
                nc.vector.scalar_tensor_tensor(
                    c_m, e1[:], c_m, c_m,
                    op0=mybir.AluOpType.mult, op1=mybir.AluOpType.add,
                )

            def epilogue_ab(m, g, ps):
                l, kind = g // 2, ("A", "B")[g % 2]
                base = m * 72
                if stage < 2:
                    return
                sq = sq_pool.tile([P, GW], _F32, tag="sq")
                nc.scalar.activation(
                    sq[:], ps[:], mybir.ActivationFunctionType.Square
                )
                slot = base + g * 8
                nc.vector.reduce_sum(
                    pg_sb[:, slot : slot + 8],
                    sq[:].rearrange("p (e r) -> p e r", e=E),
                    axis=mybir.AxisListType.X,
                )

            # scheduler-order marks: per-k wait_until timestamps force the
            # Tile scheduler to keep the emitted PE order (3-4 matmuls
            # sharing one stationary back-to-back) so dedup_ldweights can
            # drop the redundant weight loads.  Marks are scheduling-time
            # hints only; a stretched estimate keeps DMA-readiness from
            # reordering matmuls at the cost of nothing on hardware.
            stretch = float(os.environ.get("TSTRETCH", "1.25"))
            step_ms = [
                stretch * (53 + (2 * GW + GATE) / 2.4 + 30) * 1e-6,
                stretch * (53 + (4 * GW) / 2.4 + 40) * 1e-6,
            ]

            def body(_iv=None):
                weight_dmas()
                nc.vector.memset(c_sb[:], 1.0)
                t_ms = 0.0
                for m in range(mt):
                    xt_sb = xt_pool.tile([P, KT * P], mm_dtype, tag="xt")
                    nc.gpsimd.dma_start(out=xt_sb[:], in_=xtb[m])
                    if stage >= 3:
                        xp = xp_pool.tile([P, D], _F32, tag="xp")
                        nc.scalar.dma_start(
                            out=xp[:], in_=x0[m * P : (m + 1) * P, :]
                        )

                    def xk(k):
                        return xt_sb[:, k * P : (k + 1) * P]

                    for hi, half in enumerate(halves):
                        pss = []
                        for g, kind in half:
                            if kind == "G":
                                pss.append(psg_pool.tile([P, GATE], _F32,
                                                         tag="psg",
                                                         name="psg"))
                            else:
                                pss.append(ps_pool.tile([P, GW], _F32,
                                                        tag="ps",
                                                        name="ps"))
                        for k in range(KT):
                            st, sp = (k == 0), (k == KT - 1)
                            with tc.tile_wait_until(t_ms):
                                for (g, kind), ps in zip(half, pss):
                                    if kind == "G":
                                        nc.tensor.matmul(
                                            ps[:], xk(k),
                                            wg_sb[:, k * GATE :
                                                  (k + 1) * GATE],
                                            start=st, stop=sp,
                                        )
                                    else:
                                        nc.tensor.matmul(
                                            ps[:], xk(k),
                                            w_sb[g][:, k * GW :
                                                    (k + 1) * GW],
                                            start=st, stop=sp,
                                        )
                            t_ms += step_ms[hi]
                        base = m * 72
                        for (g, kind), ps in zip(half, pss):
                            if kind == "G":
                                if stage >= 2:
                                    nc.vector.tensor_copy(
                                        pg_sb[:, base + 48 : base + 72],
                                        ps[:, :GATE],
                                    )
                            else:
                                epilogue_ab(m, g, ps)
                        if stage >= 3:
                            if hi == 0:
                                recurrence(m, 0)
                            else:
                                recurrence(m, 1)
                                recurrence(m, 2)
                                c_m = c_sb[:, m : m + 1]
                                nc.vector.tensor_scalar_mul(
                                    xp[:], xp[:], c_m
                                )
                                nc.sync.dma_start(
                                    out=out[m * P : (m + 1) * P, :],
                                    in_=xp[:],
                                )
                    if stage < 3 and m == mt - 1:
                        fl = xp_pool.tile([P, GW], _F32, tag="flush")
                        nc.vector.tensor_copy(fl[:], pss[0][:])
                        nc.sync.dma_start(out=out[0:P, :GW], in_=fl[:])

            if reps == 1:
                body()
            else:
                with tc.For_i(0, reps, 1) as iv:
                    body(iv)

    if os.environ.get("DEDUP", "1") == "1":
        n = dedup_ldweights(nc)
        if os.environ.get("KERNEL_DEBUG"):
            print(f"dedup_ldweights removed {n}")
    nc.compile()
    return nc


def build_nc(mm_dtype=mybir.dt.float32r, mt: int = MT, mpp: int = MPP,
             reps: int = 1):
    if os.environ.get("SCHED", "gm") == "gi":
        return build_nc_gi(mm_dtype, mt=mt, reps=reps)
    return build_nc_gm(mm_dtype, mt=mt, mpp=mpp, reps=reps)


def build_nc_gm(mm_dtype=mybir.dt.float32r, mt: int = MT, mpp: int = MPP,
                reps: int = 1):
    """Build the per-core Bass kernel. mt<MT builds a reduced-size kernel
    (for simulation); reps>1 wraps the body in a hardware loop (timing).
    KSTAGE env: 1=matmuls only, 2=+epilogue, 3=full (default)."""
    stage = int(os.environ.get("KSTAGE", "3"))
    nc = bacc.Bacc("TRN2", target_bir_lowering=False, debug=False,
                   num_devices=N_CORES)
    bc = mt * P
    mpp = int(os.environ.get("MPP_OVR", mpp))
    whalf = os.environ.get("WHALF", "1") == "1"
    sqip = os.environ.get("SQIP", "0") == "1"
    xb = int(os.environ.get("XB", "4"))
    wb = int(os.environ.get("WB", "3"))
    mpp = min(mpp, mt)
    n_ph = (mt + mpp - 1) // mpp
    mpb = min(MPB, mpp)
    bpp = (mpp + mpb - 1) // mpb  # xt blocks per phase

    x0 = nc.dram_tensor("X0", [bc, D], _F32, kind="ExternalInput")
    xtb = nc.dram_tensor("XTB", [mt, P, KT * P], mm_dtype,
                         kind="ExternalInput")
    wab = nc.dram_tensor("WAB", [N_UV, P, KT * GW], mm_dtype,
                         kind="ExternalInput")
    wgd = nc.dram_tensor("WG", [P, KT * GATE], mm_dtype,
                         kind="ExternalInput")
    bgr = nc.dram_tensor("BG", [P, GATE], _F32, kind="ExternalInput")
    out = nc.dram_tensor("OUT", [bc, D], _F32, kind="ExternalOutput")

    # group sequence: (kind, layer); gate after B0 so the layer-0
    # recurrence can start, A/B of later layers after it.
    seq = [("A", 0), ("B", 0), ("G", 0), ("A", 1), ("B", 1),
           ("A", 2), ("B", 2)]

    with TileContext(nc) as tc:
        with (
            tc.tile_pool(name="xt_p", bufs=xb) as xt_pool,
            tc.tile_pool(name="w_p", bufs=wb) as w_pool,
            tc.tile_pool(name="wg_p", bufs=1) as wg_pool,
            tc.tile_pool(name="pg_p", bufs=1) as pg_pool,
            tc.tile_pool(name="sq_p", bufs=3) as sq_pool,
            tc.tile_pool(name="sm_p", bufs=2) as sm_pool,
            tc.tile_pool(name="xp_p", bufs=2) as xp_pool,
            tc.tile_pool(name="ps_p", bufs=int(os.environ.get("PSB", "3")),
                         space="PSUM") as ps_pool,
        ):
            # --- persistent tiles ---
            wg_sb = wg_pool.tile([P, KT * GATE], mm_dtype, tag="wg")
            bg_sb = wg_pool.tile([P, GATE], _F32, tag="bg")
            # per panel m: [pA0 pB0 pA1 pB1 pA2 pB2 (8 each) | g0 (24)]
            pg_sb = pg_pool.tile([P, mt * 72], _F32, tag="pg")
            c_sb = wg_pool.tile([P, mt], _F32, tag="c")

            nc.scalar.dma_start(out=wg_sb[:], in_=wgd[:])
            nc.scalar.dma_start(out=bg_sb[:], in_=bgr[:])

            def recurrence(m, l):
                c_m = c_sb[:, m : m + 1]
                base = m * 72
                pa = pg_sb[:, base + 2 * l * 8 : base + 2 * l * 8 + 8]
                pb = pg_sb[:, base + (2 * l + 1) * 8 : base + (2 * l + 2) * 8]
                g0l = pg_sb[:, base + 48 + l * E : base + 48 + (l + 1) * E]
                d = sm_pool.tile([P, E], _F32, tag="d")
                t = sm_pool.tile([P, E], _F32, tag="t")
                et = sm_pool.tile([P, E], _F32, tag="et")
                nmx = sm_pool.tile([P, 1], _F32, tag="nmx")
                s1 = sm_pool.tile([P, 1], _F32, tag="s1")
                s2 = sm_pool.tile([P, 1], _F32, tag="s2")
                rcp = sm_pool.tile([P, 1], _F32, tag="rcp")
                e1 = sm_pool.tile([P, 1], _F32, tag="e1")
                # d = pA - pB  (= p0[l])
                nc.vector.tensor_tensor(d[:], pa, pb,
                                        op=mybir.AluOpType.subtract)
                # t = c * g0[l] + bg[l]
                nc.vector.scalar_tensor_tensor(
                    t[:], g0l, c_m, bg_sb[:, l * E : (l + 1) * E],
                    op0=mybir.AluOpType.mult, op1=mybir.AluOpType.add,
                )
                # nmx = -max_e t
                nc.vector.tensor_reduce(
                    nmx[:], t[:], axis=mybir.AxisListType.X,
                    op=mybir.AluOpType.max, negate=True,
                )
                # et = exp(t - max); s2 = sum_e et
                nc.scalar.activation(
                    et[:], t[:], mybir.ActivationFunctionType.Exp,
                    bias=nmx[:], scale=1.0, accum_out=s2[:],
                )
                # s1 = sum_e d * et
                nc.vector.scalar_tensor_tensor(
                    t[:], d[:], 1.0, et[:],
                    op0=mybir.AluOpType.mult, op1=mybir.AluOpType.mult,
                    accum_out=s1[:],
                )
                nc.vector.reciprocal(rcp[:], s2[:])
                # e1 = s1 * rcp * c ; c += e1 * c
                nc.vector.scalar_tensor_tensor(
                    e1[:], s1[:], rcp[:], c_m,
                    op0=mybir.AluOpType.mult, op1=mybir.AluOpType.mult,
                )
                nc.vector.scalar_tensor_tensor(
                    c_m, e1[:], c_m, c_m,
                    op0=mybir.AluOpType.mult, op1=mybir.AluOpType.add,
                )

            def body(_iv=None):
                xp_tiles = {}
                nc.vector.memset(c_sb[:], 1.0)
                for ph in range(n_ph):
                    lo = ph * mpp
                    npan = min(mpp, mt - lo)
                    # phase-resident X^T blocks (plain 2D DMAs per panel)
                    blks = []
                    for b in range(bpp):
                        xt_sb = xt_pool.tile([P, mpb * KT * P], mm_dtype,
                                             tag="xt")
                        blks.append(xt_sb)
                        for j in range(mpb):
                            mloc = b * mpb + j
                            if mloc >= npan:
                                break
                            nc.gpsimd.dma_start(
                                out=xt_sb[:, j * KT * P : (j + 1) * KT * P],
                                in_=xtb[lo + mloc],
                            )

                    def xt_panel(mloc, k):
                        t = blks[mloc // mpb]
                        j = mloc % mpb
                        base = (j * KT + k) * P
                        return t[:, base : base + P]

                    for kind, l in seq:
                        if kind == "G":
                            ncols = GATE

                            def wslice(k):
                                return wg_sb[:, k * GATE : (k + 1) * GATE]
                        elif whalf:
                            g = 2 * l + (kind == "B")
                            hw = KT * GW // 2
                            whs = []
                            for h in range(2):
                                wh = w_pool.tile([P, hw], mm_dtype, tag="w")
                                nc.scalar.dma_start(
                                    out=wh[:],
                                    in_=wab[g][:, h * hw : (h + 1) * hw],
                                )
                                whs.append(wh)
                            ncols = GW

                            def wslice(k, whs=whs):
                                t = whs[k // (KT // 2)]
                                kk = k % (KT // 2)
                                return t[:, kk * GW : (kk + 1) * GW]
                        else:
                            g = 2 * l + (kind == "B")
                            w_sb = w_pool.tile([P, KT * GW], mm_dtype,
                                               tag="w")
                            nc.scalar.dma_start(out=w_sb[:], in_=wab[g])
                            ncols = GW

                            def wslice(k, w_sb=w_sb):
                                return w_sb[:, k * GW : (k + 1) * GW]
                        last = kind == "B" and l == L - 1
                        for mloc in range(npan):
                            m = lo + mloc
                            if stage >= 3 and kind == "A" and l == L - 1:
                                # prefetch X0 panel for the finalize, one
                                # group ahead of its use in B2
                                xp = xp_pool.tile([P, D], _F32, tag="xp")
                                xp_tiles[m] = xp
                                nc.gpsimd.dma_start(
                                    out=xp[:],
                                    in_=x0[m * P : (m + 1) * P, :],
                                )
                            ps = ps_pool.tile([P, GW], _F32, tag="ps")
                            for k in range(KT):
                                nc.tensor.matmul(
                                    ps[:, :ncols],
                                    xt_panel(mloc, k),
                                    wslice(k),
                                    start=(k == 0),
                                    stop=(k == KT - 1),
                                )
                            base = m * 72
                            if kind == "G":
                                if stage >= 2:
                                    nc.vector.tensor_copy(
                                        pg_sb[:, base + 48 : base + 72],
                                        ps[:, :GATE],
                                    )
                                if stage >= 3:
                                    recurrence(m, 0)
                            else:
                                if stage >= 2:
                                    if sqip:
                                        # square in place in PSUM, reduce
                                        # straight from PSUM
                                        sq = ps
                                    else:
                                        sq = sq_pool.tile([P, GW], _F32,
                                                          tag="sq")
                                    nc.scalar.activation(
                                        sq[:], ps[:],
                                        mybir.ActivationFunctionType.Square,
                                    )
                                    slot = base + (2 * l + (kind == "B")) * 8
                                    nc.vector.reduce_sum(
                                        pg_sb[:, slot : slot + 8],
                                        sq[:].rearrange(
                                            "p (e r) -> p e r", e=E
                                        ),
                                        axis=mybir.AxisListType.X,
                                    )
                                if stage >= 3 and kind == "B" and l >= 1:
                                    recurrence(m, l)
                                if stage >= 3 and last:
                                    # out panel = c * X0 panel
                                    xp = xp_tiles.pop(m)
                                    c_m = c_sb[:, m : m + 1]
                                    nc.vector.tensor_scalar_mul(
                                        xp[:], xp[:], c_m
                                    )
                                    nc.sync.dma_start(
                                        out=out[m * P : (m + 1) * P, :],
                                        in_=xp[:],
                                    )
                                if stage < 3 and last and mloc == npan - 1:
                                    fl = xp_pool.tile([P, GW], _F32,
                                                      tag="flush")
                                    nc.vector.tensor_copy(fl[:], ps[:])
                                    nc.sync.dma_start(
                                        out=out[0:P, :GW], in_=fl[:]
                                    )

            if reps == 1:
                body()
            else:
                with tc.For_i(0, reps, 1) as iv:
                    body(iv)

    nc.compile()
    return nc


def pack_weights(U, V, Wg):
    """Host-side packing: A/B groups + gate, laid out so every device DMA
    is a plain contiguous [128, N] transfer."""
    A = (U + V) * 0.5  # (L, E, R, D)
    Bm = (U - V) * 0.5
    allw = np.empty((N_UV, GW, D), np.float32)
    for l in range(L):
        allw[2 * l] = A[l].reshape(GW, D)
        allw[2 * l + 1] = Bm[l].reshape(GW, D)
    # [g, c, k, p] -> [g, p, k, c]
    wab = np.ascontiguousarray(
        allw.reshape(N_UV, GW, KT, P).transpose(0, 3, 2, 1)
    ).reshape(N_UV, P, KT * GW)
    wg = np.ascontiguousarray(
        Wg.reshape(GATE, KT, P).transpose(2, 1, 0)
    ).reshape(P, KT * GATE)
    return wab, wg


def pack_xtb(x0_shard, mt=MT):
    """[bc, D] -> [mt, P(d-within-k), KT*P(b-within-panel)] blocked
    transpose so each panel is one contiguous [128, 2048] DMA."""
    return np.ascontiguousarray(
        x0_shard.reshape(mt, P, KT, P).transpose(0, 3, 2, 1)
    ).reshape(mt, P, KT * P)


def make_in_maps(X0, U, V, Wg, bg):
    mm_np = mybir.dt.np(
        getattr(mybir.dt, os.environ.get("KERNEL_MM_DTYPE", "float32r"))
    )
    X0 = np.ascontiguousarray(np.asarray(X0, dtype=np.float32))
    wab, wg = pack_weights(
        np.asarray(U, np.float32), np.asarray(V, np.float32),
        np.asarray(Wg, np.float32)
    )
    wab = np.ascontiguousarray(wab.astype(mm_np))
    wg = np.ascontiguousarray(wg.astype(mm_np))
    bg_rep = np.ascontiguousarray(
        np.broadcast_to(np.asarray(bg, np.float32).reshape(1, GATE),
                        (P, GATE))
    )
    in_maps = []
    for c in range(N_CORES):
        sh = X0[c * BC : (c + 1) * BC]
        in_maps.append(
            {
                "X0": sh,
                "XTB": np.ascontiguousarray(pack_xtb(sh).astype(mm_np)),
                "WAB": wab,
                "WG": wg,
                "BG": bg_rep,
            }
        )
    return in_maps


_CACHE = {}


def _get_runner(mm_dtype_name: str):
    key = mm_dtype_name
    if key not in _CACHE:
        _CACHE[key] = build_nc(getattr(mybir.dt, mm_dtype_name))
    return _CACHE[key]


def kernel(X0, U, V, Wg, bg):
    in_maps = make_in_maps(X0, U, V, Wg, bg)
    mm_dtype_name = os.environ.get("KERNEL_MM_DTYPE", "float32r")
    nc = _get_runner(mm_dtype_name)
    res = run_bass_kernel_spmd(nc, in_maps, list(range(N_CORES)))
    return np.concatenate(
        [res.results[c]["OUT"] for c in range(N_CORES)], axis=0
    )

